# revision 9
# baseline (speedup 1.0000x reference)
"""BoxConv2d Trainium2 kernel.

Reference computes, per (c, f) box and batch b:
    out[b, c*FN+f, i, j] = integral of x[b, c] over the continuous window
        rows [i + x_min, i + x_max + 1) x cols [j + y_min, j + y_max + 1),
    with window coordinates clipped to [0, H] x [0, W] (bilinear sampling of
    the integral image is exact for piecewise-constant images).

That is exactly a separable band matmul with clamped-ramp overlap weights:
    Wx[i, p] = clamp01(p + 1 - (i + x_min)) - clamp01(p + 1 - (i + x_max + 1))
    Wy[j, q] = clamp01(q + 1 - (j + y_min)) - clamp01(q + 1 - (j + y_max + 1))
    out[b, cf] = Wx @ x[b, c] @ Wy^T

The Wx/Wy matrices depend only on the tiny box parameters, so they are built
on the host and shipped to the device; the device kernel is pure TensorE
matmuls in fp16 (fp32 PSUM accumulation), which numpy-validates to ~3e-4
relative error against the fp32 reference.

Sharding: channels across the 8 cores (4 channels/core, all 4 batches), box
parameters replicated per-core as part of each core's W shard.

Step 1 (x side):  V^B[j, f*256+io] = sum_p x[p, j] * Wx[f][io, p]
    lhsT (stationary) = x chunk [p-chunk, j-half], rhs = WxT [p-chunk, 2f*io].
Step 2 (y side):  out[ih*128+io, jo] = sum_j V[j, ...] * Wy[f][jo, j]
    lhsT = V chunk [j-chunk, io-half], rhs = WyT [j-chunk, jo].
"""

import numpy as np

B, C, FN, H, W = 4, 32, 4, 256, 256
N_CORES = 8
C_PER_CORE = C // N_CORES  # 4 channels per core

_PROGRAM_CACHE = {}


def _build_program():
    """Build (once) the SPMD Bass program run identically on all 8 cores."""
    import concourse.bass as bass
    import concourse.tile as tile
    from concourse import bacc, mybir

    nc = bacc.Bacc("TRN2", target_bir_lowering=False, debug=False)
    f16 = mybir.dt.float16
    f32 = mybir.dt.float32

    # Per-core inputs, host-laid-out so every DMA is one contiguous 2D copy:
    # x16[b, c, p, pc*256 + j]          = x[b, c, pc*128 + p, j]
    # wxt[c, p, (fp*2+pc)*512 + fi*256 + io] = Wx[c, 2fp+fi][io, pc*128 + p]
    # wyt[c, j, (f*2+jc)*256 + jo]      = Wy[c, f][jo, jc*128 + j]
    x16 = nc.dram_tensor("x16", [B, C_PER_CORE, 128, 512], f16,
                         kind="ExternalInput").ap()
    wxt = nc.dram_tensor("wxt", [C_PER_CORE, 128, 2048], f16,
                         kind="ExternalInput").ap()
    wyt = nc.dram_tensor("wyt", [C_PER_CORE, 128, 2048], f16,
                         kind="ExternalInput").ap()
    # out_dev[b, c, p, f*512 + a*256 + jo] = out[b, c*FN+f, a*128+p, jo]
    # (host transposes back; keeps store DMAs fully contiguous per partition)
    out = nc.dram_tensor("out", [B, C_PER_CORE, 128, 2048], f32,
                         kind="ExternalOutput").ap()

    with tile.TileContext(nc) as tc:
        with (
            tc.tile_pool(name="wx", bufs=2) as wx_pool,
            tc.tile_pool(name="wy", bufs=2) as wy_pool,
            tc.tile_pool(name="xin", bufs=4) as x_pool,
            tc.tile_pool(name="v", bufs=4) as v_pool,
            tc.tile_pool(name="osb", bufs=3) as o_pool,
            tc.tile_pool(name="psv", bufs=2, space=bass.MemorySpace.PSUM) as psv_pool,
            tc.tile_pool(name="pso", bufs=4, space=bass.MemorySpace.PSUM) as pso_pool,
        ):
            xt0 = None
            for c in range(C_PER_CORE):
                # First x tile + first Wx chunk are on the critical path:
                # issue on separate engines/queues, Wx 4-way-split in MM use
                # order so the first matmul (subtile deps) waits only on the
                # first 128KB.  (Per-queue DMA BW is ~110GB/s.)
                if c == 0:
                    xt0 = x_pool.tile([128, 512], f16, tag="x", name="x")
                    nc.gpsimd.dma_start(xt0[:], x16[0, 0])
                wx_t = wx_pool.tile([128, 2048], f16, tag="wx", name="wx")
                nsplit = 4 if c == 0 else 2
                weng = nc.sync if c == 0 else nc.gpsimd
                step = 2048 // nsplit
                for q in range(nsplit):
                    weng.dma_start(wx_t[:, q * step:(q + 1) * step],
                                   wxt[c][:, q * step:(q + 1) * step])
                wy_t = wy_pool.tile([128, 2048], f16, tag="wy", name="wy")
                for q in range(2):
                    nc.gpsimd.dma_start(wy_t[:, q * 1024:(q + 1) * 1024],
                                        wyt[c][:, q * 1024:(q + 1) * 1024])

                for b in range(B):
                    if c == 0 and b == 0:
                        xt = xt0
                    else:
                        xt = x_pool.tile([128, 512], f16, tag="x", name="x")
                        nc.gpsimd.dma_start(xt[:], x16[b, c])

                    # Step 1: psv holds both f-pairs (2 PSUM banks); one
                    # big PSUM->SBUF cast per jh, alternating engine.
                    vt = [v_pool.tile([128, 1024], f16, tag="v", name="v")
                          for _jh in range(2)]
                    for jh in range(2):
                        psv = psv_pool.tile([128, 1024], f32, tag="psv",
                                            name="psv")
                        for fp in range(2):
                            for pc in range(2):
                                nc.tensor.matmul(
                                    psv[:, fp * 512:(fp + 1) * 512],
                                    xt[:, pc * 256 + jh * 128:
                                       pc * 256 + jh * 128 + 128],
                                    wx_t[:, (fp * 2 + pc) * 512:
                                         (fp * 2 + pc) * 512 + 512],
                                    start=(pc == 0),
                                    stop=(pc == 1),
                                )
                        eng = nc.vector.tensor_copy if jh == 0 else nc.scalar.copy
                        eng(vt[jh][:], psv[:])

                    # Step 2
                    osb = o_pool.tile([128, 2048], f32, tag="o", name="osb")
                    for f in range(FN):
                        pso = pso_pool.tile([128, 512], f32, tag="pso",
                                            name="pso")
                        for ih in range(2):
                            for jc in range(2):
                                nc.tensor.matmul(
                                    pso[:, ih * 256:(ih + 1) * 256],
                                    vt[jc][:, f * 256 + ih * 128:
                                           f * 256 + ih * 128 + 128],
                                    wy_t[:, (f * 2 + jc) * 256:
                                         (f * 2 + jc) * 256 + 256],
                                    start=(jc == 0),
                                    stop=(jc == 1),
                                )
                        dst = osb[:, f * 512:(f + 1) * 512]
                        eng = nc.vector.tensor_copy if f % 2 == 0 else nc.scalar.copy
                        eng(dst[:], pso[:])
                    # contiguous store, split across queues and engines;
                    # 4-way on the final tile to shorten the kernel tail
                    last = (c == C_PER_CORE - 1 and b == B - 1)
                    nsp = 4 if last else 2
                    stp = 2048 // nsp
                    for q in range(nsp):
                        seng = nc.sync if q % 2 == 0 else nc.gpsimd
                        seng.dma_start(out[b, c][:, q * stp:(q + 1) * stp],
                                       osb[:, q * stp:(q + 1) * stp])

    nc.compile()
    return nc


def _get_program():
    if "nc" not in _PROGRAM_CACHE:
        _PROGRAM_CACHE["nc"] = _build_program()
    return _PROGRAM_CACHE["nc"]


def _band(mn, mx, dim):
    """Overlap weights W[i, p] of clipped window [i+mn, i+mx+1) with cell
    [p, p+1), built in fp64."""
    i = np.arange(dim, dtype=np.float64)[:, None]
    p = np.arange(dim, dtype=np.float64)[None, :]
    lo = i + float(mn)
    hi = i + float(mx) + 1.0
    return np.clip(p + 1.0 - lo, 0.0, 1.0) - np.clip(p + 1.0 - hi, 0.0, 1.0)


def _prepare_in_maps(input, x_min, x_max, y_min, y_max):
    # x16[b, c, p, pc*256 + j] = x[b, c, pc*128 + p, j]
    x16_full = np.ascontiguousarray(
        input.astype(np.float16).reshape(B, C, 2, 128, 256)
        .transpose(0, 1, 3, 2, 4).reshape(B, C, 128, 512))

    in_maps = []
    for core in range(N_CORES):
        c0 = core * C_PER_CORE
        wxt = np.empty((C_PER_CORE, 128, 2048), dtype=np.float16)
        wyt = np.empty((C_PER_CORE, 128, 2048), dtype=np.float16)
        for cl in range(C_PER_CORE):
            c = c0 + cl
            for f in range(FN):
                WxT = _band(x_min[c, f], x_max[c, f], H).T.astype(np.float16)
                WyT = _band(y_min[c, f], y_max[c, f], W).T.astype(np.float16)
                fp, fi = f // 2, f % 2
                for pc in range(2):
                    base = (fp * 2 + pc) * 512 + fi * 256
                    wxt[cl, :, base:base + 256] = WxT[pc * 128:(pc + 1) * 128]
                for jc in range(2):
                    base = (f * 2 + jc) * 256
                    wyt[cl, :, base:base + 256] = WyT[jc * 128:(jc + 1) * 128]
        in_maps.append({
            "x16": np.ascontiguousarray(x16_full[:, c0:c0 + C_PER_CORE]),
            "wxt": wxt,
            "wyt": wyt,
        })
    return in_maps


def run(input, x_min, x_max, y_min, y_max, trace=False):
    """Run the SPMD kernel; returns (full_output, BassKernelResults)."""
    from concourse.bass_utils import run_bass_kernel_spmd

    nc = _get_program()
    in_maps = _prepare_in_maps(
        np.asarray(input, dtype=np.float32),
        np.asarray(x_min, dtype=np.float64),
        np.asarray(x_max, dtype=np.float64),
        np.asarray(y_min, dtype=np.float64),
        np.asarray(y_max, dtype=np.float64),
    )
    res = run_bass_kernel_spmd(nc, in_maps, list(range(N_CORES)), trace=trace)
    # out_dev[b, c, p, f*512 + a*256 + jo] -> out[b, c*FN+f, a*128+p, jo]
    parts = []
    for i in range(N_CORES):
        o = res.results[i]["out"].reshape(B, C_PER_CORE, 128, FN, 2, 256)
        parts.append(o.transpose(0, 1, 3, 4, 2, 5).reshape(
            B, C_PER_CORE * FN, 256, 256))
    full = np.ascontiguousarray(np.concatenate(parts, axis=1))
    return full, res


def kernel(input, x_min, x_max, y_min, y_max):
    full, _ = run(input, x_min, x_max, y_min, y_max)
    return full


# revision 10
# speedup vs baseline: 1.1653x; 1.1653x over previous
"""BoxConv2d Trainium2 kernel.

Reference computes, per (c, f) box and batch b:
    out[b, c*FN+f, i, j] = integral of x[b, c] over the continuous window
        rows [i + x_min, i + x_max + 1) x cols [j + y_min, j + y_max + 1),
    with window coordinates clipped to [0, H] x [0, W] (bilinear sampling of
    the integral image is exact for piecewise-constant images).

That is exactly a separable band matmul with clamped-ramp overlap weights:
    Wx[i, p] = clamp01(p + 1 - (i + x_min)) - clamp01(p + 1 - (i + x_max + 1))
    Wy[j, q] = clamp01(q + 1 - (j + y_min)) - clamp01(q + 1 - (j + y_max + 1))
    out[b, cf] = Wx @ x[b, c] @ Wy^T

The Wx/Wy matrices depend only on the tiny box parameters, so they are built
on the host and shipped to the device; the device kernel is pure TensorE
matmuls in fp16 (fp32 PSUM accumulation), which numpy-validates to ~3e-4
relative error against the fp32 reference.

Sharding: channels across the 8 cores (4 channels/core, all 4 batches), box
parameters replicated per-core as part of each core's W shard.

Step 1 (x side):  V^B[j, f*256+io] = sum_p x[p, j] * Wx[f][io, p]
    lhsT (stationary) = x chunk [p-chunk, j-half], rhs = WxT [p-chunk, 2f*io].
Step 2 (y side):  out[ih*128+io, jo] = sum_j V[j, ...] * Wy[f][jo, j]
    lhsT = V chunk [j-chunk, io-half], rhs = WyT [j-chunk, jo].
"""

import numpy as np

B, C, FN, H, W = 4, 32, 4, 256, 256
N_CORES = 8
C_PER_CORE = C // N_CORES  # 4 channels per core

_PROGRAM_CACHE = {}


def _build_program():
    """Build (once) the SPMD Bass program run identically on all 8 cores."""
    import concourse.bass as bass
    import concourse.tile as tile
    from concourse import bacc, mybir

    nc = bacc.Bacc("TRN2", target_bir_lowering=False, debug=False)
    f16 = mybir.dt.float16
    f32 = mybir.dt.float32

    # Per-core inputs, host-laid-out so every DMA is one contiguous 2D copy:
    # x16[b, c, p, pc*256 + j]          = x[b, c, pc*128 + p, j]
    # wxt[c, p, (fp*2+pc)*512 + fi*256 + io] = Wx[c, 2fp+fi][io, pc*128 + p]
    # wyt[c, j, (f*2+jc)*256 + jo]      = Wy[c, f][jo, jc*128 + j]
    x16 = nc.dram_tensor("x16", [B, C_PER_CORE, 128, 512], f16,
                         kind="ExternalInput").ap()
    wxt = nc.dram_tensor("wxt", [C_PER_CORE, 128, 2048], f16,
                         kind="ExternalInput").ap()
    wyt = nc.dram_tensor("wyt", [C_PER_CORE, 128, 2048], f16,
                         kind="ExternalInput").ap()
    # out_dev[b, c, p, f*512 + a*256 + jo] = out[b, c*FN+f, a*128+p, jo]
    # (host transposes back; keeps store DMAs fully contiguous per partition)
    out = nc.dram_tensor("out", [B, C_PER_CORE, 128, 2048], f32,
                         kind="ExternalOutput").ap()

    with tile.TileContext(nc) as tc:
        with (
            tc.tile_pool(name="wx", bufs=2) as wx_pool,
            tc.tile_pool(name="wy", bufs=2) as wy_pool,
            tc.tile_pool(name="xin", bufs=4) as x_pool,
            tc.tile_pool(name="v", bufs=4) as v_pool,
            tc.tile_pool(name="osb", bufs=3) as o_pool,
            tc.tile_pool(name="psv", bufs=2, space=bass.MemorySpace.PSUM) as psv_pool,
            tc.tile_pool(name="pso", bufs=4, space=bass.MemorySpace.PSUM) as pso_pool,
        ):
            xt0 = None
            for c in range(C_PER_CORE):
                # First x tile + first Wx chunk are on the critical path:
                # issue on separate engines/queues, Wx 4-way-split in MM use
                # order so the first matmul (subtile deps) waits only on the
                # first 128KB.  (Per-queue DMA BW is ~110GB/s.)
                if c == 0:
                    xt0 = x_pool.tile([128, 512], f16, tag="x", name="x")
                    nc.gpsimd.dma_start(xt0[:], x16[0, 0])
                wx_t = wx_pool.tile([128, 2048], f16, tag="wx", name="wx")
                nsplit = 4 if c == 0 else 2
                step = 2048 // nsplit
                for q in range(nsplit):
                    nc.gpsimd.dma_start(wx_t[:, q * step:(q + 1) * step],
                                        wxt[c][:, q * step:(q + 1) * step])
                wy_t = wy_pool.tile([128, 2048], f16, tag="wy", name="wy")
                for q in range(2):
                    nc.gpsimd.dma_start(wy_t[:, q * 1024:(q + 1) * 1024],
                                        wyt[c][:, q * 1024:(q + 1) * 1024])

                for b in range(B):
                    if c == 0 and b == 0:
                        xt = xt0
                    else:
                        xt = x_pool.tile([128, 512], f16, tag="x", name="x")
                        nc.gpsimd.dma_start(xt[:], x16[b, c])

                    # Step 1: psv holds both f-pairs (2 PSUM banks); one
                    # big PSUM->SBUF cast per jh, alternating engine.
                    vt = [v_pool.tile([128, 1024], f16, tag="v", name="v")
                          for _jh in range(2)]
                    for jh in range(2):
                        psv = psv_pool.tile([128, 1024], f32, tag="psv",
                                            name="psv")
                        for fp in range(2):
                            for pc in range(2):
                                nc.tensor.matmul(
                                    psv[:, fp * 512:(fp + 1) * 512],
                                    xt[:, pc * 256 + jh * 128:
                                       pc * 256 + jh * 128 + 128],
                                    wx_t[:, (fp * 2 + pc) * 512:
                                         (fp * 2 + pc) * 512 + 512],
                                    start=(pc == 0),
                                    stop=(pc == 1),
                                )
                        eng = nc.vector.tensor_copy if jh == 0 else nc.scalar.copy
                        eng(vt[jh][:], psv[:])

                    # Step 2
                    osb = o_pool.tile([128, 2048], f32, tag="o", name="osb")
                    for f in range(FN):
                        pso = pso_pool.tile([128, 512], f32, tag="pso",
                                            name="pso")
                        for ih in range(2):
                            for jc in range(2):
                                nc.tensor.matmul(
                                    pso[:, ih * 256:(ih + 1) * 256],
                                    vt[jc][:, f * 256 + ih * 128:
                                           f * 256 + ih * 128 + 128],
                                    wy_t[:, (f * 2 + jc) * 256:
                                         (f * 2 + jc) * 256 + 256],
                                    start=(jc == 0),
                                    stop=(jc == 1),
                                )
                        dst = osb[:, f * 512:(f + 1) * 512]
                        eng = nc.vector.tensor_copy if f % 2 == 0 else nc.scalar.copy
                        eng(dst[:], pso[:])
                    # contiguous store on sync, split across DMA queues;
                    # 4-way on the final tile to shorten the kernel tail
                    last = (c == C_PER_CORE - 1 and b == B - 1)
                    nsp = 4 if last else 2
                    stp = 2048 // nsp
                    for q in range(nsp):
                        nc.sync.dma_start(out[b, c][:, q * stp:(q + 1) * stp],
                                          osb[:, q * stp:(q + 1) * stp])

    nc.compile()
    return nc


def _get_program():
    if "nc" not in _PROGRAM_CACHE:
        _PROGRAM_CACHE["nc"] = _build_program()
    return _PROGRAM_CACHE["nc"]


def _band(mn, mx, dim):
    """Overlap weights W[i, p] of clipped window [i+mn, i+mx+1) with cell
    [p, p+1), built in fp64."""
    i = np.arange(dim, dtype=np.float64)[:, None]
    p = np.arange(dim, dtype=np.float64)[None, :]
    lo = i + float(mn)
    hi = i + float(mx) + 1.0
    return np.clip(p + 1.0 - lo, 0.0, 1.0) - np.clip(p + 1.0 - hi, 0.0, 1.0)


def _prepare_in_maps(input, x_min, x_max, y_min, y_max):
    # x16[b, c, p, pc*256 + j] = x[b, c, pc*128 + p, j]
    x16_full = np.ascontiguousarray(
        input.astype(np.float16).reshape(B, C, 2, 128, 256)
        .transpose(0, 1, 3, 2, 4).reshape(B, C, 128, 512))

    in_maps = []
    for core in range(N_CORES):
        c0 = core * C_PER_CORE
        wxt = np.empty((C_PER_CORE, 128, 2048), dtype=np.float16)
        wyt = np.empty((C_PER_CORE, 128, 2048), dtype=np.float16)
        for cl in range(C_PER_CORE):
            c = c0 + cl
            for f in range(FN):
                WxT = _band(x_min[c, f], x_max[c, f], H).T.astype(np.float16)
                WyT = _band(y_min[c, f], y_max[c, f], W).T.astype(np.float16)
                fp, fi = f // 2, f % 2
                for pc in range(2):
                    base = (fp * 2 + pc) * 512 + fi * 256
                    wxt[cl, :, base:base + 256] = WxT[pc * 128:(pc + 1) * 128]
                for jc in range(2):
                    base = (f * 2 + jc) * 256
                    wyt[cl, :, base:base + 256] = WyT[jc * 128:(jc + 1) * 128]
        in_maps.append({
            "x16": np.ascontiguousarray(x16_full[:, c0:c0 + C_PER_CORE]),
            "wxt": wxt,
            "wyt": wyt,
        })
    return in_maps


def run(input, x_min, x_max, y_min, y_max, trace=False):
    """Run the SPMD kernel; returns (full_output, BassKernelResults)."""
    from concourse.bass_utils import run_bass_kernel_spmd

    nc = _get_program()
    in_maps = _prepare_in_maps(
        np.asarray(input, dtype=np.float32),
        np.asarray(x_min, dtype=np.float64),
        np.asarray(x_max, dtype=np.float64),
        np.asarray(y_min, dtype=np.float64),
        np.asarray(y_max, dtype=np.float64),
    )
    res = run_bass_kernel_spmd(nc, in_maps, list(range(N_CORES)), trace=trace)
    # out_dev[b, c, p, f*512 + a*256 + jo] -> out[b, c*FN+f, a*128+p, jo]
    parts = []
    for i in range(N_CORES):
        o = res.results[i]["out"].reshape(B, C_PER_CORE, 128, FN, 2, 256)
        parts.append(o.transpose(0, 1, 3, 4, 2, 5).reshape(
            B, C_PER_CORE * FN, 256, 256))
    full = np.ascontiguousarray(np.concatenate(parts, axis=1))
    return full, res


def kernel(input, x_min, x_max, y_min, y_max):
    full, _ = run(input, x_min, x_max, y_min, y_max)
    return full
